# revision 14
# baseline (speedup 1.0000x reference)
"""Trainium2 Bass kernel for a 12-head causal attention block (B=4, T=2048, C=768).

Sharding: 8 cores = 4 batches x 2 head-groups (6 heads each). Each core computes
q/k/v projections for its head-group over its batch's full sequence, causal
flash-style attention, and a partial output projection (row-parallel Wp).
Host sums the two partial outputs per batch. No cross-core collectives.

Per-core layouts (channel-major to keep everything PE-friendly, no transposes):
  xT   [768, 2048]  x[b].T
  wq/wk/wv [768, 384]  W[g*384:(g+1)*384, :].T   (lhsT layout [c_in, c_out])
  wp   [384, 768]  Wp[:, g*384:(g+1)*384].T      (lhsT layout [j, c_out])
  masks [4, 128, 512] causal mask tiles for the diagonal blocks
  out yT [768, 2048] partial (attn_out_group @ Wp_group.T).T
"""

import numpy as np

T = 2048
C = 768
G = 384          # channels per head-group (6 heads x 64)
DH = 64
NK = C // 128    # 6 k-tiles over c_in
TBLK = 512
NTB = T // TBLK  # 4 t-blocks
NST = T // 128   # 16 s-tiles
N_CORES = 8

_CACHE = {}


def _emit(tc, yT, xT, wq, wk, wv, wp, masks, dbg=None):
    import concourse.mybir as mybir

    nc = tc.nc
    DT = mybir.dt.float32
    R = mybir.dt.float32r
    Exp = mybir.ActivationFunctionType.Exp

    def mm(out, lhsT, rhs, start, stop, skip=False):
        nc.tensor.matmul(out, lhsT=lhsT, rhs=rhs,
                         start=start, stop=stop, skip_group_check=skip)

    with (
        tc.tile_pool(name="pc", bufs=1) as pc,        # persistent sbuf
        tc.tile_pool(name="px", bufs=2) as px,        # x chunks
        tc.tile_pool(name="pe", bufs=3) as pe,        # exp tiles
        tc.tile_pool(name="pr", bufs=2) as pr,        # recip + y-out staging
        tc.tile_pool(name="pao", bufs=2) as pao,      # attn-out per t-block
        tc.tile_pool(name="pdram", bufs=4, space="DRAM") as pdram,  # denom roundtrip
        tc.tile_pool(name="psA", bufs=2, space="PSUM") as psA,    # proj matmuls
        tc.tile_pool(name="ps01", bufs=2, space="PSUM") as ps01p,  # scores
        tc.tile_pool(name="pso", bufs=1, space="PSUM") as pso,    # attn accum
        tc.tile_pool(name="psd", bufs=1, space="PSUM") as psd,    # denom accum
    ):
        # ---- persistent tensors ----
        wq_sb = pc.tile([128, NK * G], R, tag="wq")
        wk_sb = pc.tile([128, NK * G], R, tag="wk")
        wv_sb = pc.tile([128, NK * G], R, tag="wv")
        for k in range(NK):
            nc.sync.dma_start(out=wq_sb[:, k * G:(k + 1) * G], in_=wq[k * 128:(k + 1) * 128, :])
            nc.sync.dma_start(out=wk_sb[:, k * G:(k + 1) * G], in_=wk[k * 128:(k + 1) * 128, :])
            nc.sync.dma_start(out=wv_sb[:, k * G:(k + 1) * G], in_=wv[k * 128:(k + 1) * 128, :])
        wp_sb = pc.tile([128, 3 * C], R, tag="wp")
        for kk in range(3):
            nc.sync.dma_start(out=wp_sb[:, kk * C:(kk + 1) * C], in_=wp[kk * 128:(kk + 1) * 128, :])
        masks_sb = pc.tile([128, 4, TBLK], DT, tag="masks")
        for o in range(4):
            nc.sync.dma_start(out=masks_sb[:, o, :], in_=masks[o])

        # qT/kT: [128, 3*2048]; channel c of group -> partition c%128, block c//128.
        # head h (0..5): partitions (h%2)*64..+64 of block h//2.
        qT_sb = pc.tile([128, 3 * T], R, tag="qT")
        kT_sb = pc.tile([128, 3 * T], R, tag="kT")
        # v token-major, augmented: per (st, h) slot of 65 cols at (st*6+h)*65:
        # cols 0-63 = v[st*128 + p, h*64 + m], col 64 = 1.0 (denominator row)
        v_sb = pc.tile([128, NST * 6 * 65], R, tag="v")
        ones_f = pc.tile([128, NST * 6], DT, tag="ones_f")
        nc.vector.memset(ones_f[:], 1.0)
        nc.vector.tensor_copy(
            out=v_sb.rearrange("p (s c) -> p s c", c=65)[:, :, 64], in_=ones_f[:])

        for tb in range(NTB):
            # ---- phase 1 for this t-block: q/k/v projections ----
            xc = []
            for k in range(NK):
                t_ = px.tile([128, TBLK], R, tag=f"xc{k}")
                nc.sync.dma_start(out=t_[:], in_=xT[k * 128:(k + 1) * 128, tb * TBLK:(tb + 1) * TBLK])
                xc.append(t_)
            for w_sb, out_sb in ((wq_sb, qT_sb), (wk_sb, kT_sb)):
                for mo in range(3):
                    ps = psA.tile([128, TBLK], DT, tag="mm512")
                    for k in range(NK):
                        mm(ps[:], w_sb[:, k * G + mo * 128: k * G + (mo + 1) * 128],
                           xc[k][:], start=(k == 0), stop=(k == NK - 1))
                    nc.vector.tensor_copy(
                        out=out_sb[:, mo * T + tb * TBLK: mo * T + (tb + 1) * TBLK], in_=ps[:])
            for sl in range(4):
                st = 4 * tb + sl
                ps = psA.tile([128, TBLK], DT, tag="mm512")
                for k in range(NK):
                    mm(ps[:, 0:G], xc[k][:, sl * 128:(sl + 1) * 128],
                       wv_sb[:, k * G:(k + 1) * G], start=(k == 0), stop=(k == NK - 1))
                for h in range(6):
                    nc.vector.tensor_copy(
                        out=v_sb[:, (st * 6 + h) * 65:(st * 6 + h) * 65 + 64],
                        in_=ps[:, h * 64:(h + 1) * 64])

            if dbg is not None and tb == 0:
                nc.sync.dma_start(out=dbg["q"][0], in_=qT_sb[:].bitcast(DT))
                nc.sync.dma_start(out=dbg["k"][0], in_=kT_sb[:].bitcast(DT))
                nc.sync.dma_start(out=dbg["v"][0], in_=v_sb[:].bitcast(DT))

            # ---- phase 2: attention for this t-block, head pairs j ----
            ao = pao.tile([128, 3 * TBLK], R, tag="ao")
            n_st = 4 * (tb + 1)
            for j in range(3):
                poA = pso.tile([65, TBLK], DT, tag="poA")
                poB = psd.tile([65, TBLK], DT, tag="poB")
                qs = qT_sb[:, j * T + tb * TBLK: j * T + (tb + 1) * TBLK]
                for st in range(n_st):
                    ks = kT_sb[:, j * T + st * 128: j * T + st * 128 + 128]
                    ps = ps01p.tile([128, 2 * TBLK], DT, tag="ps01")
                    mm(ps[:, 0:TBLK], ks[0:64, :], qs[0:64, :], start=True, stop=True)
                    mm(ps[:, TBLK:2 * TBLK], ks[64:128, :], qs[64:128, :], start=True, stop=True)
                    e01 = pe.tile([128, 2 * TBLK], R, tag="e01")
                    nc.scalar.activation(out=e01[:], in_=ps[:], func=Exp, scale=float(DH) ** -0.5)
                    if dbg is not None and tb == 0 and j == 0:
                        sdump = pr.tile([128, 2 * TBLK], DT, tag="sdump")
                        nc.vector.tensor_copy(out=sdump[:], in_=ps[:])
                        nc.sync.dma_start(out=dbg["sc"][st], in_=sdump[:])
                    if st >= 4 * tb:  # diagonal block: causal mask
                        o = st - 4 * tb
                        w_ = 128 * (o + 1)
                        nc.vector.tensor_mul(e01[:, 0:w_], e01[:, 0:w_], masks_sb[:, o, 0:w_])
                        nc.vector.tensor_mul(e01[:, TBLK:TBLK + w_], e01[:, TBLK:TBLK + w_],
                                             masks_sb[:, o, 0:w_])
                    if dbg is not None and tb == 0:
                        nc.sync.dma_start(out=dbg["e"][j * 4 + st], in_=e01[:].bitcast(DT))
                    first, last = st == 0, st == n_st - 1
                    e0 = e01[:, 0:TBLK]
                    e1 = e01[:, TBLK:2 * TBLK]
                    s0 = (st * 6 + 2 * j) * 65
                    s1 = (st * 6 + 2 * j + 1) * 65
                    mm(poA[:, :], v_sb[:, s0:s0 + 65], e0, start=first, stop=last)
                    mm(poB[:, :], v_sb[:, s1:s1 + 65], e1, start=first, stop=last)
                # normalize: rows 0-63 = attn, row 64 = denominator
                tA = pr.tile([65, TBLK], DT, tag="tA")
                tB = pr.tile([65, TBLK], DT, tag="tB")
                nc.vector.tensor_copy(out=tA[:], in_=poA[:])
                nc.vector.tensor_copy(out=tB[:], in_=poB[:])
                d01 = pr.tile([128, TBLK], DT, tag="d01")
                tS = pr.tile([128, TBLK], DT, tag="tS")
                dsc = pdram.tile([2, TBLK], DT, tag="dsc")
                nc.sync.dma_start(out=dsc[0:1, :], in_=tA[64:65, :])
                nc.sync.dma_start(out=dsc[1:2, :], in_=tB[64:65, :])
                nc.sync.dma_start(out=d01[0:64, :], in_=dsc[0:1, :].to_broadcast([64, TBLK]))
                nc.sync.dma_start(out=d01[64:128, :], in_=dsc[1:2, :].to_broadcast([64, TBLK]))
                nc.sync.dma_start(out=tS[64:128, :], in_=tB[0:64, :])
                rd = pr.tile([128, TBLK], DT, tag="rd")
                nc.vector.reciprocal(out=rd[:], in_=d01[:])
                nc.vector.tensor_mul(ao[0:64, j * TBLK:(j + 1) * TBLK], tA[0:64, :], rd[0:64, :])
                nc.vector.tensor_mul(ao[64:128, j * TBLK:(j + 1) * TBLK], tS[64:128, :], rd[64:128, :])
                if dbg is not None and tb == 0:
                    nc.sync.dma_start(out=dbg["tA"][j], in_=tA[:])
                    nc.sync.dma_start(out=dbg["tB"][j], in_=tB[:])
                    nc.sync.dma_start(out=dbg["d01"][j], in_=d01[:])
                    nc.sync.dma_start(out=dbg["rd"][j], in_=rd[:])
                    nc.sync.dma_start(out=dbg["tS"][j], in_=tS[64:128, :])

            # ---- phase 3: partial output projection for this t-block ----
            for mo in range(6):
                py = psA.tile([128, TBLK], DT, tag="mm512")
                for kk in range(3):
                    mm(py[:], wp_sb[:, kk * C + mo * 128: kk * C + (mo + 1) * 128],
                       ao[:, kk * TBLK:(kk + 1) * TBLK], start=(kk == 0), stop=(kk == 2))
                yo = pr.tile([128, TBLK], DT, tag="yo")
                nc.vector.tensor_copy(out=yo[:], in_=py[:])
                nc.sync.dma_start(out=yT[mo * 128:(mo + 1) * 128, tb * TBLK:(tb + 1) * TBLK],
                                  in_=yo[:])


def build_program():
    if "nc" in _CACHE:
        return _CACHE["nc"]
    import concourse.bacc as bacc
    import concourse.tile as tile
    import concourse.mybir as mybir

    nc = bacc.Bacc("TRN2", target_bir_lowering=False, debug=False)
    DT = mybir.dt.float32
    R = mybir.dt.float32r
    xT_d = nc.dram_tensor("xT", [C, T], R, kind="ExternalInput")
    wq_d = nc.dram_tensor("wq", [C, G], R, kind="ExternalInput")
    wk_d = nc.dram_tensor("wk", [C, G], R, kind="ExternalInput")
    wv_d = nc.dram_tensor("wv", [C, G], R, kind="ExternalInput")
    wp_d = nc.dram_tensor("wp", [G, C], R, kind="ExternalInput")
    mk_d = nc.dram_tensor("masks", [4, 128, TBLK], DT, kind="ExternalInput")
    yT_d = nc.dram_tensor("yT", [C, T], DT, kind="ExternalOutput")

    with tile.TileContext(nc) as tc:
        _emit(tc, yT_d.ap(), xT_d.ap(), wq_d.ap(), wk_d.ap(), wv_d.ap(),
              wp_d.ap(), mk_d.ap())
    nc.compile()
    _CACHE["nc"] = nc
    return nc


def make_masks():
    s = np.arange(128)[:, None]
    t = np.arange(TBLK)[None, :]
    return np.stack([(t >= 128 * o + s) for o in range(4)]).astype(np.float32)


def shard_inputs(x, Wq, Wk, Wv, Wp):
    """Full inputs -> list of 8 per-core input dicts."""
    x = np.asarray(x, dtype=np.float32)
    Wq, Wk, Wv, Wp = (np.asarray(w, dtype=np.float32) for w in (Wq, Wk, Wv, Wp))
    masks = make_masks()
    in_maps = []
    for c in range(N_CORES):
        b, g = divmod(c, 2)
        sl = slice(g * G, (g + 1) * G)
        in_maps.append({
            "xT": np.ascontiguousarray(x[b].T),
            "wq": np.ascontiguousarray(Wq[sl, :].T),
            "wk": np.ascontiguousarray(Wk[sl, :].T),
            "wv": np.ascontiguousarray(Wv[sl, :].T),
            "wp": np.ascontiguousarray(Wp[:, sl].T),
            "masks": masks,
        })
    return in_maps


def combine_outputs(results):
    """Per-core {'yT': [768,2048]} partials -> full [4, 2048, 768] output."""
    out = np.empty((4, T, C), dtype=np.float32)
    for b in range(4):
        acc = results[2 * b]["yT"] + results[2 * b + 1]["yT"]
        out[b] = acc.T
    return out


def kernel(x, Wq, Wk, Wv, Wp, **run_kwargs):
    from concourse.bass_utils import run_bass_kernel_spmd

    nc = build_program()
    in_maps = shard_inputs(x, Wq, Wk, Wv, Wp)
    res = run_bass_kernel_spmd(nc, in_maps, core_ids=list(range(N_CORES)), **run_kwargs)
    out = combine_outputs(res.results)
    if run_kwargs:
        return out, res
    return out


# revision 16
# speedup vs baseline: 1.2485x; 1.2485x over previous
"""Trainium2 Bass kernel for a 12-head causal attention block (B=4, T=2048, C=768).

Sharding: 8 cores = 4 batches x 2 head-groups (6 heads each). Each core computes
q/k/v projections for its head-group over its batch's full sequence, causal
flash-style attention, and a partial output projection (row-parallel Wp).
Host sums the two partial outputs per batch. No cross-core collectives.

All matmul operands are fp16 (fp32 PSUM accumulation); measured end-to-end
relative error ~7e-4 vs the fp32 reference. Layouts are channel-major so no
on-chip transposes are needed:
  xT   [768, 2048]  x[b].T                        (fp16)
  wq/wk/wv [768, 384]  W[g*384:(g+1)*384, :].T    (fp16, lhsT layout)
  wp   [384, 768]  Wp[:, g*384:(g+1)*384].T       (fp16, lhsT layout)
  masks [4, 128, 512] causal mask tiles, ones [128, 64]   (fp16)
  out yT [768, 2048] fp32 partial = (attn_out_group @ Wp_group.T).T
"""

import numpy as np

T = 2048
C = 768
G = 384          # channels per head-group (6 heads x 64)
DH = 64
NK = C // 128    # 6 k-tiles over c_in
TBLK = 512
NTB = T // TBLK  # 4 t-blocks
NST = T // 128   # 16 s-tiles
N_CORES = 8

_CACHE = {}


def _emit(tc, yT, xT, wq, wk, wv, wp, masks, ones, dbg=None):
    import concourse.mybir as mybir

    nc = tc.nc
    DT = mybir.dt.float32
    H = mybir.dt.float16
    Exp = mybir.ActivationFunctionType.Exp
    mm = nc.tensor.matmul

    with (
        tc.tile_pool(name="pc", bufs=1) as pc,        # persistent sbuf
        tc.tile_pool(name="px", bufs=2) as px,        # x chunks
        tc.tile_pool(name="pe", bufs=4) as pe,        # exp tiles
        tc.tile_pool(name="pr", bufs=3) as pr,        # recip + y-out staging
        tc.tile_pool(name="pao", bufs=2) as pao,      # attn-out per t-block
        tc.tile_pool(name="psP", bufs=2, space="PSUM") as psP,    # proj + scores
        tc.tile_pool(name="pso", bufs=2, space="PSUM") as pso,    # attn accum
        tc.tile_pool(name="psd", bufs=2, space="PSUM") as psd,    # denom accum
    ):
        # ---- persistent tensors ----
        wq_sb = pc.tile([128, NK * G], H, tag="wq")
        wk_sb = pc.tile([128, NK * G], H, tag="wk")
        wv_sb = pc.tile([128, NK * G], H, tag="wv")
        for k in range(NK):
            nc.sync.dma_start(out=wq_sb[:, k * G:(k + 1) * G], in_=wq[k * 128:(k + 1) * 128, :])
            nc.sync.dma_start(out=wk_sb[:, k * G:(k + 1) * G], in_=wk[k * 128:(k + 1) * 128, :])
            nc.sync.dma_start(out=wv_sb[:, k * G:(k + 1) * G], in_=wv[k * 128:(k + 1) * 128, :])
        wp_sb = pc.tile([128, 3 * C], H, tag="wp")
        for kk in range(3):
            nc.sync.dma_start(out=wp_sb[:, kk * C:(kk + 1) * C], in_=wp[kk * 128:(kk + 1) * 128, :])
        masks_sb = pc.tile([128, 4, TBLK], H, tag="masks")
        for o in range(4):
            nc.sync.dma_start(out=masks_sb[:, o, :], in_=masks[o])
        ones_sb = pc.tile([128, 64], H, tag="ones")
        nc.sync.dma_start(out=ones_sb[:], in_=ones)

        # qT/kT: [128, 3*2048]; channel c of group -> partition c%128, block c//128.
        # head h (0..5): partitions (h%2)*64..+64 of block h//2.
        qT_sb = pc.tile([128, 3 * T], H, tag="qT")
        kT_sb = pc.tile([128, 3 * T], H, tag="kT")
        # v token-major: [128, 16*384]; col (st*6+h)*64 + m = v[st*128 + p, h*64 + m]
        v_sb = pc.tile([128, NST * G], H, tag="v")

        for tb in range(NTB):
            # ---- phase 1 for this t-block: q/k/v projections ----
            xc = []
            for k in range(NK):
                t_ = px.tile([128, TBLK], H, tag=f"xc{k}")
                nc.sync.dma_start(out=t_[:], in_=xT[k * 128:(k + 1) * 128, tb * TBLK:(tb + 1) * TBLK])
                xc.append(t_)
            for w_sb, out_sb in ((wq_sb, qT_sb), (wk_sb, kT_sb)):
                for mo in range(3):
                    ps = psP.tile([128, 2 * TBLK], DT, tag="pp")
                    for k in range(NK):
                        mm(ps[:, 0:TBLK], lhsT=w_sb[:, k * G + mo * 128: k * G + (mo + 1) * 128],
                           rhs=xc[k][:], start=(k == 0), stop=(k == NK - 1))
                    nc.vector.tensor_copy(
                        out=out_sb[:, mo * T + tb * TBLK: mo * T + (tb + 1) * TBLK],
                        in_=ps[:, 0:TBLK])
            for sl in range(4):
                st = 4 * tb + sl
                ps = psP.tile([128, 2 * TBLK], DT, tag="pp")
                for k in range(NK):
                    mm(ps[:, 0:G], lhsT=xc[k][:, sl * 128:(sl + 1) * 128],
                       rhs=wv_sb[:, k * G:(k + 1) * G], start=(k == 0), stop=(k == NK - 1))
                nc.vector.tensor_copy(out=v_sb[:, st * G:(st + 1) * G], in_=ps[:, 0:G])

            # ---- phase 2: attention for this t-block, head pairs j ----
            ao = pao.tile([128, 3 * TBLK], H, tag="ao")
            n_st = 4 * (tb + 1)
            for j in range(3):
                po = pso.tile([128, TBLK], DT, tag="po")
                pd = psd.tile([128, TBLK], DT, tag="pd")
                qs = qT_sb[:, j * T + tb * TBLK: j * T + (tb + 1) * TBLK]

                def pv_group(st, e01, first, last):
                    e0 = e01[:, 0:TBLK]
                    e1 = e01[:, TBLK:2 * TBLK]
                    s0 = (st * 6 + 2 * j) * DH
                    s1 = (st * 6 + 2 * j + 1) * DH
                    mm(po[0:64, :], lhsT=v_sb[:, s0:s0 + DH], rhs=e0,
                       start=first, stop=last, skip_group_check=True)
                    mm(po[64:128, :], lhsT=v_sb[:, s1:s1 + DH], rhs=e1,
                       start=first, stop=last, skip_group_check=True)
                    mm(pd[0:64, :], lhsT=ones_sb[:], rhs=e0,
                       start=first, stop=last, skip_group_check=True)
                    mm(pd[64:128, :], lhsT=ones_sb[:], rhs=e1,
                       start=first, stop=last, skip_group_check=True)

                pending = None  # software pipeline: PV(st-1) is issued after scores(st)
                for st in range(n_st):
                    ks = kT_sb[:, j * T + st * 128: j * T + st * 128 + 128]
                    ps = psP.tile([128, 2 * TBLK], DT, tag="pp")
                    mm(ps[:, 0:TBLK], lhsT=ks[0:64, :], rhs=qs[0:64, :], start=True, stop=True)
                    mm(ps[:, TBLK:2 * TBLK], lhsT=ks[64:128, :], rhs=qs[64:128, :], start=True, stop=True)
                    e01 = pe.tile([128, 2 * TBLK], H, tag="e01")
                    nc.scalar.activation(out=e01[:], in_=ps[:], func=Exp, scale=float(DH) ** -0.5)
                    if st >= 4 * tb:  # diagonal block: causal mask
                        o = st - 4 * tb
                        w_ = 128 * (o + 1)
                        nc.vector.tensor_mul(e01[:, 0:w_], e01[:, 0:w_], masks_sb[:, o, 0:w_])
                        nc.vector.tensor_mul(e01[:, TBLK:TBLK + w_], e01[:, TBLK:TBLK + w_],
                                             masks_sb[:, o, 0:w_])
                    if pending is not None:
                        pv_group(pending[0], pending[1], pending[0] == 0, False)
                    pending = (st, e01)
                pv_group(pending[0], pending[1], pending[0] == 0, True)
                rd = pr.tile([128, TBLK], DT, tag="rd")
                nc.vector.reciprocal(out=rd[:], in_=pd[:])
                nc.vector.tensor_mul(ao[:, j * TBLK:(j + 1) * TBLK], po[:], rd[:])

            # ---- phase 3: partial output projection for this t-block ----
            for mo in range(6):
                py = psP.tile([128, 2 * TBLK], DT, tag="pp")
                for kk in range(3):
                    mm(py[:, 0:TBLK], lhsT=wp_sb[:, kk * C + mo * 128: kk * C + (mo + 1) * 128],
                       rhs=ao[:, kk * TBLK:(kk + 1) * TBLK], start=(kk == 0), stop=(kk == 2))
                yo = pr.tile([128, TBLK], DT, tag="yo")
                nc.vector.tensor_copy(out=yo[:], in_=py[:, 0:TBLK])
                nc.sync.dma_start(out=yT[mo * 128:(mo + 1) * 128, tb * TBLK:(tb + 1) * TBLK],
                                  in_=yo[:])


def build_program():
    if "nc" in _CACHE:
        return _CACHE["nc"]
    import concourse.bacc as bacc
    import concourse.tile as tile
    import concourse.mybir as mybir

    nc = bacc.Bacc("TRN2", target_bir_lowering=False, debug=False)
    DT = mybir.dt.float32
    H = mybir.dt.float16
    xT_d = nc.dram_tensor("xT", [C, T], H, kind="ExternalInput")
    wq_d = nc.dram_tensor("wq", [C, G], H, kind="ExternalInput")
    wk_d = nc.dram_tensor("wk", [C, G], H, kind="ExternalInput")
    wv_d = nc.dram_tensor("wv", [C, G], H, kind="ExternalInput")
    wp_d = nc.dram_tensor("wp", [G, C], H, kind="ExternalInput")
    mk_d = nc.dram_tensor("masks", [4, 128, TBLK], H, kind="ExternalInput")
    on_d = nc.dram_tensor("ones", [128, 64], H, kind="ExternalInput")
    yT_d = nc.dram_tensor("yT", [C, T], DT, kind="ExternalOutput")

    with tile.TileContext(nc) as tc:
        _emit(tc, yT_d.ap(), xT_d.ap(), wq_d.ap(), wk_d.ap(), wv_d.ap(),
              wp_d.ap(), mk_d.ap(), on_d.ap())
    nc.compile()
    _CACHE["nc"] = nc
    return nc


def make_masks():
    s = np.arange(128)[:, None]
    t = np.arange(TBLK)[None, :]
    return np.stack([(t >= 128 * o + s) for o in range(4)]).astype(np.float16)


def shard_inputs(x, Wq, Wk, Wv, Wp):
    """Full inputs -> list of 8 per-core input dicts (fp16 operands)."""
    x = np.asarray(x, dtype=np.float32)
    Wq, Wk, Wv, Wp = (np.asarray(w, dtype=np.float32) for w in (Wq, Wk, Wv, Wp))
    masks = make_masks()
    ones = np.ones((128, 64), dtype=np.float16)
    in_maps = []
    for c in range(N_CORES):
        b, g = divmod(c, 2)
        sl = slice(g * G, (g + 1) * G)
        in_maps.append({
            "xT": np.ascontiguousarray(x[b].T).astype(np.float16),
            "wq": np.ascontiguousarray(Wq[sl, :].T).astype(np.float16),
            "wk": np.ascontiguousarray(Wk[sl, :].T).astype(np.float16),
            "wv": np.ascontiguousarray(Wv[sl, :].T).astype(np.float16),
            "wp": np.ascontiguousarray(Wp[:, sl].T).astype(np.float16),
            "masks": masks,
            "ones": ones,
        })
    return in_maps


def combine_outputs(results):
    """Per-core {'yT': [768,2048]} partials -> full [4, 2048, 768] output."""
    out = np.empty((4, T, C), dtype=np.float32)
    for b in range(4):
        acc = results[2 * b]["yT"] + results[2 * b + 1]["yT"]
        out[b] = acc.T
    return out


def kernel(x, Wq, Wk, Wv, Wp, **run_kwargs):
    from concourse.bass_utils import run_bass_kernel_spmd

    nc = build_program()
    in_maps = shard_inputs(x, Wq, Wk, Wv, Wp)
    res = run_bass_kernel_spmd(nc, in_maps, core_ids=list(range(N_CORES)), **run_kwargs)
    out = combine_outputs(res.results)
    if run_kwargs:
        return out, res
    return out
